# revision 26
# baseline (speedup 1.0000x reference)
"""Trainium2 Bass kernel for GCNNetwork (GENConv message passing, L=6).

Graph-data parallel over 8 NeuronCores; v2 (fp16 compute pipeline).

 - Nodes sharded contiguously: core c owns rows [7500c, 7500c+7500), re-binned
   into 61 blocks of 128 slots so every block has <=256 in-edges (2 edge tiles).
 - h and y live in SBUF for the whole kernel (fp16); only y is spilled to DRAM
   for the AllGather (split in two halves so the first overlaps LN of the
   second half of blocks).
 - Per edge-tile-pair (one block): one indirect gather of y[src] ([128,2,256]
   fp16), msg = relu(ysrc + ea*wlw) on Act, w = exp(msg-4) on Act (shift keeps
   fp16 in range; cancels in the softmax ratio), ev1 = msg*w on DVE, then a
   0/1-indicator matmul accumulates [denom | numer] into PSUM (fp16 inputs,
   fp32 accumulate).
 - Block post: agg = numer/max(denom,1e-3) + y (DVE divide), transpose, conv
   matmul (fp16 weights), relu -> h (SBUF), pooling indicator matmul.
 - Pool windows are scattered into a [3200,256] fp16 z buffer, AllReduced,
   readout MLP computed replicated on every core in fp16 (fp32 accumulates).

All weights replicated; indicator matrices (st/ind), initial embeddings h0,
and fp16 weight copies are precomputed on the host.
"""
import sys
import numpy as np

for _p in ("/opt/trn_rl_repo", "/root/.axon_site/_ro/trn_rl_repo"):
    if _p not in sys.path:
        sys.path.append(_p)

import concourse.bass as bass
import concourse.bacc as bacc
import concourse.mybir as mybir
import concourse.tile as tile
from concourse.bass_utils import run_bass_kernel_spmd

F32 = mybir.dt.float32
F16 = mybir.dt.float16
I32 = mybir.dt.int32
ALU = mybir.AluOpType
ACTF = mybir.ActivationFunctionType

N, E, B, D, L = 60000, 120000, 512, 256, 6
NTYPES = 25
LN_EPS = 1e-5
EXP_SHIFT = -4.0          # w = exp(msg + EXP_SHIFT); cancels in softmax ratio
DEN_CLAMP = 1e-4          # ln(denom + eps) bias; << exp(EXP_SHIFT) = 0.018
NC = 8
NPC = N // NC             # 7500 real nodes per core
NBLK = 61                 # 128-slot node blocks per core (bin-packed)
NSHARD = NBLK * 128       # 7808 slots per core
TPB = 2                   # edge tiles per block (asserted in prep)
T = NBLK * TPB            # 122 edge tiles per core
GRP = 4                   # blocks per indirect-gather group
ZROWS = 3200              # z buffer rows (L*512 = 3072 used, row 3072 = dump)
ZDUMP = 3072

# module-level knobs (test.py pokes these; harness uses defaults)
TRACE = False
TRACE_CORES = None
LAST_RESULT = {}
CCE_ADD = True            # fuse ea*wlw + y[src] via gather compute_op
DEBUG = False             # add layer-0 intermediate dumps as outputs

_prog_cache = {}


# ----------------------------------------------------------------------------
# host-side preprocessing
# ----------------------------------------------------------------------------

def _glob_row(slot):
    """Global slot -> row in y_full (plain concatenation by core)."""
    return slot


def _prep(inputs):
    x = np.asarray(inputs["x"]).astype(np.int32).reshape(-1)
    ei = np.asarray(inputs["edge_index"]).astype(np.int64)
    ea = np.asarray(inputs["edge_attr"]).astype(np.float32).reshape(-1)
    batch = np.asarray(inputs["batch"]).astype(np.int64).reshape(-1)
    src_all, dst_all = ei[0], ei[1]

    # ---- pass 1: per-core node permutation (bin-pack by in-degree) ----
    glob_slot = np.zeros(N, dtype=np.int64)        # node id -> global slot
    slot_node = []                                 # per core: slot -> node id
    bin_cnts = np.zeros((NC, NBLK), dtype=np.int64)
    for c in range(NC):
        lo, hi = c * NPC, (c + 1) * NPC
        deg = np.bincount(dst_all[(dst_all >= lo) & (dst_all < hi)] - lo,
                          minlength=NPC)
        order = np.argsort(-deg, kind="stable")    # local ids, degree desc
        i = np.arange(NPC)
        chunk, pos = i // NBLK, i % NBLK
        bins = np.where(chunk % 2 == 0, pos, NBLK - 1 - pos)
        cnt = np.zeros(NBLK, dtype=np.int64)
        for b in range(NBLK):
            cnt[b] = deg[order[bins == b]].sum()
        bin_cnts[c] = cnt
        sn = np.full(NSHARD, -1, dtype=np.int64)
        sl = bins * 128 + chunk                    # slot per order-position
        sn[sl] = order + lo
        slot_node.append(sn)
        loc_slot = np.empty(NPC, dtype=np.int64)
        loc_slot[order] = sl
        glob_slot[lo:hi] = c * NSHARD + loc_slot

    assert bin_cnts.max() <= TPB * 128, f"block overflow: {bin_cnts.max()}"

    # ---- pass 2: per-core edge arrays / node arrays in slot order ----
    in_maps = []
    dst_slot_all = glob_slot[dst_all]              # global slot of dst
    node_emb = np.asarray(inputs["node_emb"]).astype(np.float32)
    h0_all = node_emb[x]                           # [N, D]
    colidx = np.arange(128, dtype=np.int64)
    for c in range(NC):
        sel = (dst_slot_all >= c * NSHARD) & (dst_slot_all < (c + 1) * NSHARD)
        ds = dst_slot_all[sel] - c * NSHARD        # local slot of dst
        s = src_all[sel]
        a = ea[sel]
        blk = ds >> 7
        o = np.argsort(blk, kind="stable")
        ds, s, a, blk = ds[o], s[o], a[o], blk[o]
        cnts = np.bincount(blk, minlength=NBLK)
        assert np.all(cnts == bin_cnts[c])
        ne = len(ds)
        bstart = np.concatenate([[0], np.cumsum(cnts)])
        rank = np.arange(ne) - np.repeat(bstart[:-1], cnts)
        slot = blk * (TPB * 128) + rank            # edge slot (tile-major)
        esrc = np.zeros(T * 128, dtype=np.int32)
        dstl = np.full(T * 128, -1, dtype=np.int64)
        eav = np.zeros(T * 128, dtype=np.float32)
        esrc[slot] = _glob_row(glob_slot[s]).astype(np.int32)
        dstl[slot] = ds & 127
        eav[slot] = a
        esrc_pm = esrc.reshape(T, 128).T.copy()
        ea_pm = eav.reshape(T, 128).T.copy()
        # st indicator [128 edge, T, 128]: st[p, t, q] = (dstl[t*128+p]==q)
        dst_pm = dstl.reshape(T, 128).T            # [128, T]
        st_pm = (dst_pm[:, :, None] == colidx[None, None, :]).astype(
            np.float16).copy()

        # ---- node arrays in slot order ----
        sn = slot_node[c]
        valid = sn >= 0
        g0 = int(batch[c * NPC])
        bl = np.full(NSHARD, -1, dtype=np.int64)
        bl[valid] = batch[sn[valid]] - g0
        assert bl.max() < 128, "graph window exceeds 128 per core"
        ind_pm = (bl.reshape(NBLK, 128).T[:, :, None] ==
                  colidx[None, None, :]).astype(np.float16).copy()
        h0 = np.zeros((NSHARD, D), dtype=np.float16)
        h0[valid] = h0_all[sn[valid]].astype(np.float16)
        h0_pm = h0.reshape(NBLK, 128, D).transpose(1, 0, 2).copy()
        zrow_pm = np.zeros((128, L), dtype=np.int32)
        g = g0 + np.arange(128)
        for i in range(L):
            zrow_pm[:, i] = np.where(g < B, 512 * i + g, ZDUMP)

        in_maps.append(
            dict(esrc=esrc_pm, eaf=ea_pm, st=st_pm, ind=ind_pm,
                 h0=h0_pm, zrow=zrow_pm)
        )

    # ---- shared weights (fp16) ----
    wl_w = np.asarray(inputs["wl_w"]).astype(np.float32)      # [L,1,D]
    conv_w = np.asarray(inputs["conv_w"]).astype(np.float32)  # [L,D,D]
    ln_scale = np.asarray(inputs["ln_scale"]).astype(np.float32)
    ln_bias = np.asarray(inputs["ln_bias"]).astype(np.float32)
    wl_b = np.asarray(inputs["wl_b"]).astype(np.float32)
    conv_b = np.asarray(inputs["conv_b"]).astype(np.float32)
    ro_w = [np.asarray(inputs[f"ro_w{i}"]).astype(np.float32) for i in range(4)]
    ro_b = [np.asarray(inputs[f"ro_b{i}"]).astype(np.float32) for i in range(4)]

    flags = dict(
        ln_affine=not (np.all(ln_scale == 1.0) and np.all(ln_bias == 0.0)),
        wl_b=bool(np.any(wl_b != 0.0)),
        conv_b=bool(np.any(conv_b != 0.0)),
        ro_b=any(np.any(b != 0.0) for b in ro_b),
    )

    shared = dict(
        wlw=np.repeat(wl_w.reshape(L, 1, D), 128, axis=1).astype(np.float16),
        convw=conv_w.astype(np.float16),
        ident=np.eye(128, dtype=np.float16),
        row0=ro_w[0].astype(np.float16), row1=ro_w[1].astype(np.float16),
        row2=ro_w[2].astype(np.float16), row3=ro_w[3].astype(np.float16),
    )
    if flags["ln_affine"]:
        shared["lnsc"] = np.repeat(ln_scale.reshape(L, 1, D), 128, axis=1).copy()
        shared["lnbs"] = np.repeat(ln_bias.reshape(L, 1, D), 128, axis=1).copy()
    if flags["wl_b"]:
        shared["wlb"] = np.repeat(wl_b.reshape(L, 1, D), 128, axis=1).astype(
            np.float16)
    if flags["conv_b"]:
        shared["convb"] = np.repeat(conv_b.reshape(L, 1, D), 128, axis=1).copy()
    if flags["ro_b"]:
        for i, b_ in enumerate(ro_b):
            shared[f"rob{i}"] = np.repeat(b_.reshape(1, -1), 128, axis=0).copy()

    for m in in_maps:
        m.update(shared)
    return in_maps, flags


# ----------------------------------------------------------------------------
# device program
# ----------------------------------------------------------------------------

def _build(flags):
    nc = bacc.Bacc("TRN2", target_bir_lowering=False, debug=False,
                   num_devices=NC)

    # const APs for activation float biases
    for tag, val in (("lneps", LN_EPS), ("eshift", EXP_SHIFT),
                     ("denc", DEN_CLAMP)):
        t = nc.alloc_sbuf_tensor(f"const-float32-{tag}", [128, 1], F32)
        nc.gpsimd.memset(t.ap(), val)
        nc.const_aps.aps[(F32, val)] = t.ap()
    nc.all_engine_barrier()

    # inputs
    esrc = nc.dram_tensor("esrc", [128, T], I32, kind="ExternalInput")
    eaf = nc.dram_tensor("eaf", [128, T], F32, kind="ExternalInput")
    st = nc.dram_tensor("st", [128, T, 128], F16, kind="ExternalInput")
    ind = nc.dram_tensor("ind", [128, NBLK, 128], F16, kind="ExternalInput")
    h0 = nc.dram_tensor("h0", [128, NBLK, D], F16, kind="ExternalInput")
    zrow = nc.dram_tensor("zrow", [128, L], I32, kind="ExternalInput")
    wlw = nc.dram_tensor("wlw", [L, 128, D], F16, kind="ExternalInput")
    convw = nc.dram_tensor("convw", [L, D, D], F16, kind="ExternalInput")
    ident = nc.dram_tensor("ident", [128, 128], F16, kind="ExternalInput")
    row0 = nc.dram_tensor("row0", [6 * D, 768], F16, kind="ExternalInput")
    row1 = nc.dram_tensor("row1", [768, 384], F16, kind="ExternalInput")
    row2 = nc.dram_tensor("row2", [384, 192], F16, kind="ExternalInput")
    row3 = nc.dram_tensor("row3", [192, 1], F16, kind="ExternalInput")
    lnsc = lnbs = wlb = convb = None
    if flags["ln_affine"]:
        lnsc = nc.dram_tensor("lnsc", [L, 128, D], F32, kind="ExternalInput")
        lnbs = nc.dram_tensor("lnbs", [L, 128, D], F32, kind="ExternalInput")
    if flags["wl_b"]:
        wlb = nc.dram_tensor("wlb", [L, 128, D], F16, kind="ExternalInput")
    if flags["conv_b"]:
        convb = nc.dram_tensor("convb", [L, 128, D], F32, kind="ExternalInput")
    robs = None
    if flags["ro_b"]:
        robs = [
            nc.dram_tensor(f"rob{i}", [128, n], F32, kind="ExternalInput")
            for i, n in enumerate([768, 384, 192, 1])
        ]

    out = nc.dram_tensor("out", [B, 1], F32, kind="ExternalOutput")
    if DEBUG:
        dbg_y = nc.dram_tensor("dbg_y", [128, NBLK, D], F16,
                               kind="ExternalOutput")
        dbg_pre = nc.dram_tensor("dbg_pre", [128, GRP * TPB, D], F16,
                                 kind="ExternalOutput")
        dbg_ev = nc.dram_tensor("dbg_ev", [128, GRP * TPB, 2 * D], F16,
                                kind="ExternalOutput")
        dbg_h = nc.dram_tensor("dbg_h", [128, NBLK, D], F16,
                               kind="ExternalOutput")

    with tile.TileContext(nc) as tc:
        with (
            tc.tile_pool(name="dram", bufs=1, space="DRAM") as dram,
            tc.tile_pool(name="consts", bufs=1) as cpool,
            tc.tile_pool(name="lweights", bufs=2) as wpool,
        ):
            y_c = dram.tile([NSHARD, D], F16, tag="y_c")
            y_fulls = [
                dram.tile([NC * NSHARD, D], F16, tag=f"y_full_{i}",
                          name=f"y_full_{i}", addr_space="Shared")
                for i in range(L)
            ]
            z_all = dram.tile([ZROWS, D], F16, tag="z_all")
            z_red = dram.tile([ZROWS, D], F16, tag="z_red", addr_space="Shared")

            ident_t = cpool.tile([128, 128], F16, tag="ident")
            nc.sync.dma_start(out=ident_t[:], in_=ident[:])
            esrc_sb = cpool.tile([128, T], I32, tag="esrc_sb")
            nc.sync.dma_start(out=esrc_sb[:], in_=esrc[:])
            ea_sb = cpool.tile([128, T], F32, tag="ea_sb")
            nc.sync.dma_start(out=ea_sb[:], in_=eaf[:])
            zrow_sb = cpool.tile([128, L], I32, tag="zrow_sb")
            nc.sync.dma_start(out=zrow_sb[:], in_=zrow[:])
            st_sb = cpool.tile([128, T, 128], F16, tag="st_sb")
            nc.sync.dma_start(out=st_sb[:], in_=st[:])
            ind_sb = cpool.tile([128, NBLK, 128], F16, tag="ind_sb")
            nc.sync.dma_start(out=ind_sb[:], in_=ind[:])
            h_sb = cpool.tile([128, NBLK, D], F16, tag="h_sb")
            nc.sync.dma_start(out=h_sb[:], in_=h0[:])
            y_sb = cpool.tile([128, NBLK, D], F16, tag="y_sb")

            # ---------- zero z_all ----------
            with tc.tile_pool(name="zz", bufs=1) as zz:
                zt = zz.tile([128, ZROWS // 128, D], F16)
                nc.vector.memset(zt[:], 0.0)
                nc.sync.dma_start(
                    out=z_all[:].rearrange("(k p) d -> p k d", p=128),
                    in_=zt[:],
                )

            # ---------- layers ----------
            with (
                tc.tile_pool(name="ln", bufs=3) as lp,
                tc.tile_pool(name="edge", bufs=4) as xp,
                tc.tile_pool(name="blk", bufs=3) as bp,
                tc.tile_pool(name="ps_nd", bufs=2, space="PSUM") as ps_nd,
                tc.tile_pool(name="ps_xt", bufs=2, space="PSUM") as ps_xt,
                tc.tile_pool(name="ps_h", bufs=2, space="PSUM") as ps_h,
                tc.tile_pool(name="ps_pool", bufs=1, space="PSUM") as ps_pool,
            ):
                for li in range(L):
                    wlw_t = wpool.tile([128, D], F16, tag="wlw")
                    nc.sync.dma_start(out=wlw_t[:], in_=wlw[li])
                    cw_t = wpool.tile([128, 2, D], F16, tag="cw")
                    nc.sync.dma_start(
                        out=cw_t[:],
                        in_=convw[li].rearrange("(c p) d -> p c d", p=128),
                    )
                    if flags["ln_affine"]:
                        lnsc_t = wpool.tile([128, D], F32, tag="lnsc")
                        nc.sync.dma_start(out=lnsc_t[:], in_=lnsc[li])
                        lnbs_t = wpool.tile([128, D], F32, tag="lnbs")
                        nc.sync.dma_start(out=lnbs_t[:], in_=lnbs[li])
                    if flags["wl_b"]:
                        wlb_t = wpool.tile([128, D], F16, tag="wlb")
                        nc.sync.dma_start(out=wlb_t[:], in_=wlb[li])
                    if flags["conv_b"]:
                        convb_t = wpool.tile([128, D], F32, tag="convb")
                        nc.sync.dma_start(out=convb_t[:], in_=convb[li])

                    # ---- LayerNorm: h_sb -> y_sb (and y_c for AllGather) ----
                    for b in range(NBLK):
                        stats = lp.tile([128, 6], F32, tag="stats")
                        nc.vector.bn_stats(stats[:], h_sb[:, b, :])
                        aggr = lp.tile([128, 2], F32, tag="aggr")
                        nc.vector.bn_aggr(aggr[:], stats[:])
                        lnv = lp.tile([128, 1], F32, tag="lnv")
                        nc.scalar.activation(
                            lnv[:], aggr[:, 1:2], ACTF.Ln, bias=LN_EPS
                        )
                        rs = lp.tile([128, 1], F32, tag="rs")
                        nc.scalar.activation(
                            rs[:], lnv[:], ACTF.Exp, scale=-0.5
                        )
                        nc.vector.tensor_scalar(
                            out=y_sb[:, b, :], in0=h_sb[:, b, :],
                            scalar1=aggr[:, 0:1], scalar2=rs[:],
                            op0=ALU.subtract, op1=ALU.mult,
                        )
                        if flags["ln_affine"]:
                            nc.vector.tensor_tensor(
                                out=y_sb[:, b, :], in0=y_sb[:, b, :],
                                in1=lnsc_t[:], op=ALU.mult,
                            )
                            nc.vector.tensor_tensor(
                                out=y_sb[:, b, :], in0=y_sb[:, b, :],
                                in1=lnbs_t[:], op=ALU.add,
                            )
                        # spill y to DRAM in 4 batches for the collective
                        if b in (15, 30, 45, 60):
                            lo = {15: 0, 30: 16, 45: 31, 60: 46}[b]
                            rows = slice(lo * 128, (b + 1) * 128)
                            nc.sync.dma_start(
                                out=y_c[rows, :].rearrange(
                                    "(j p) d -> p j d", p=128),
                                in_=y_sb[:, lo:b + 1, :],
                            )

                    if DEBUG and li == 0:
                        nc.sync.dma_start(out=dbg_y[:], in_=y_sb[:])

                    # ---- AllGather y ----
                    y_full = y_fulls[li]
                    nc.gpsimd.collective_compute(
                        "AllGather", ALU.bypass,
                        replica_groups=[list(range(NC))],
                        ins=[y_c[:].opt()],
                        outs=[y_full[:].opt()],
                    )

                    # ---- edges + conv + pool (groups of GRP blocks) ----
                    ppool = ps_pool.tile([128, D], F32, tag="ppool")
                    for g0 in range(0, NBLK, GRP):
                        gn = min(GRP, NBLK - g0)
                        nt = gn * TPB
                        # pre = ea*wlw, then gather-accumulate y[src] on top
                        pre = xp.tile([128, GRP * TPB, D], F16, tag="pre")
                        for k in range(nt):
                            tt = TPB * g0 + k
                            nc.vector.tensor_scalar(
                                out=pre[:, k, :], in0=wlw_t[:],
                                scalar1=ea_sb[:, tt:tt + 1], scalar2=None,
                                op0=ALU.mult,
                            )
                        if flags["wl_b"]:
                            nc.vector.tensor_tensor(
                                out=pre[:, :nt, :], in0=pre[:, :nt, :],
                                in1=wlb_t[:, None, :].to_broadcast(
                                    [128, nt, D]),
                                op=ALU.add,
                            )
                        ysrc = xp.tile([128, GRP * TPB, D], F16, tag="ysrc")
                        for k in range(nt):
                            tt = TPB * g0 + k
                            nc.gpsimd.indirect_dma_start(
                                out=ysrc[:, k, :], out_offset=None,
                                in_=y_full[:],
                                in_offset=bass.IndirectOffsetOnAxis(
                                    ap=esrc_sb[:, tt:tt + 1], axis=0,
                                ),
                            )
                        nc.vector.tensor_tensor(
                            out=pre[:, :nt, :], in0=pre[:, :nt, :],
                            in1=ysrc[:, :nt, :], op=ALU.add,
                        )
                        msg = xp.tile([128, GRP * TPB, D], F16, tag="msg")
                        nc.vector.tensor_scalar(
                            out=msg[:, :nt, :], in0=pre[:, :nt, :],
                            scalar1=0.0, scalar2=None, op0=ALU.max,
                        )
                        ev = xp.tile([128, GRP * TPB, 2 * D], F16, tag="ev")
                        nc.scalar.activation(
                            ev[:, :nt, 0:D], msg[:, :nt, :], ACTF.Exp,
                            bias=EXP_SHIFT,
                        )
                        nc.vector.tensor_tensor(
                            out=ev[:, :nt, D:2 * D], in0=msg[:, :nt, :],
                            in1=ev[:, :nt, 0:D], op=ALU.mult,
                        )
                        if DEBUG and li == 0 and g0 == 0:
                            nc.sync.dma_start(out=dbg_pre[:], in_=pre[:])
                            nc.sync.dma_start(out=dbg_ev[:], in_=ev[:])
                        for k in range(gn):
                            b = g0 + k
                            nd = ps_nd.tile([128, 2 * D], F32, tag="nd")
                            for j in range(TPB):
                                nc.tensor.matmul(
                                    out=nd[:],
                                    lhsT=st_sb[:, TPB * b + j, :],
                                    rhs=ev[:, TPB * k + j, :],
                                    start=(j == 0), stop=(j == TPB - 1),
                                )
                            # block post: agg = numer/denom + y, conv, pool
                            # 1/denom via exp(-ln(denom + eps)) on Act
                            num16 = bp.tile([128, D], F16, tag="num16")
                            nc.scalar.activation(
                                num16[:], nd[:, D:2 * D], ACTF.Copy)
                            lnd = bp.tile([128, D], F16, tag="lnd")
                            nc.scalar.activation(
                                lnd[:], nd[:, 0:D], ACTF.Ln, bias=DEN_CLAMP)
                            rec = bp.tile([128, D], F16, tag="rec")
                            nc.scalar.activation(
                                rec[:], lnd[:], ACTF.Exp, scale=-1.0)
                            xv = bp.tile([128, D], F16, tag="xv")
                            nc.vector.tensor_tensor(
                                out=xv[:], in0=num16[:], in1=rec[:],
                                op=ALU.mult,
                            )
                            xva = bp.tile([128, D], F16, tag="xva")
                            nc.vector.tensor_tensor(
                                out=xva[:], in0=xv[:], in1=y_sb[:, b, :],
                                op=ALU.add,
                            )
                            pxt = ps_xt.tile([128, D], F16, tag="pxt")
                            nc.tensor.transpose(
                                out=pxt[:, 0:128], in_=xva[:, 0:128],
                                identity=ident_t[:],
                            )
                            nc.tensor.transpose(
                                out=pxt[:, 128:256], in_=xva[:, 128:256],
                                identity=ident_t[:],
                            )
                            xts = bp.tile([128, D], F16, tag="xts")
                            nc.vector.tensor_copy(out=xts[:], in_=pxt[:])
                            ph = ps_h.tile([128, D], F32, tag="ph")
                            for c in range(2):
                                nc.tensor.matmul(
                                    out=ph[:],
                                    lhsT=xts[:, 128 * c:128 * (c + 1)],
                                    rhs=cw_t[:, c, :],
                                    start=(c == 0), stop=(c == 1),
                                )
                            if flags["conv_b"]:
                                nc.vector.tensor_tensor(
                                    out=ph[:], in0=ph[:], in1=convb_t[:],
                                    op=ALU.add,
                                )
                            nc.scalar.activation(
                                h_sb[:, b, :], ph[:], ACTF.Relu)
                            nc.tensor.matmul(
                                out=ppool[:], lhsT=ind_sb[:, b, :],
                                rhs=h_sb[:, b, :],
                                start=(b == 0), stop=(b == NBLK - 1),
                            )
                    if DEBUG and li == 0:
                        nc.sync.dma_start(out=dbg_h[:], in_=h_sb[:])
                    # pool -> z_all
                    zp = bp.tile([128, D], F16, tag="zp")
                    nc.scalar.activation(zp[:], ppool[:], ACTF.Copy)
                    nc.gpsimd.indirect_dma_start(
                        out=z_all[:],
                        out_offset=bass.IndirectOffsetOnAxis(
                            ap=zrow_sb[:, li:li + 1], axis=0
                        ),
                        in_=zp[:], in_offset=None,
                    )

            # ---------- AllReduce z ----------
            nc.gpsimd.collective_compute(
                "AllReduce", ALU.add,
                replica_groups=[list(range(NC))],
                ins=[z_all[:].opt()], outs=[z_red[:].opt()],
            )

            # ---------- readout MLP (replicated, fp16) ----------
            with (
                tc.tile_pool(name="row", bufs=1) as rw,
                tc.tile_pool(name="ro", bufs=2) as ro,
                tc.tile_pool(name="ps_a", bufs=1, space="PSUM") as psa,
                tc.tile_pool(name="ps_b", bufs=1, space="PSUM") as psb,
                tc.tile_pool(name="ps_t", bufs=2, space="PSUM") as pst,
                tc.tile_pool(name="ps_o", bufs=1, space="PSUM") as pso,
            ):
                w0t = []
                for f in range(12):
                    w = rw.tile([128, 768], F16, tag=f"w0_{f}")
                    nc.sync.dma_start(out=w[:], in_=row0[f * 128:(f + 1) * 128, :])
                    w0t.append(w)
                w1t = []
                for f in range(6):
                    w = rw.tile([128, 384], F16, tag=f"w1_{f}")
                    nc.sync.dma_start(out=w[:], in_=row1[f * 128:(f + 1) * 128, :])
                    w1t.append(w)
                w2t = []
                for f in range(3):
                    w = rw.tile([128, 192], F16, tag=f"w2_{f}")
                    nc.sync.dma_start(out=w[:], in_=row2[f * 128:(f + 1) * 128, :])
                    w2t.append(w)
                w3a = rw.tile([128, 1], F16, tag="w3a")
                nc.sync.dma_start(out=w3a[:], in_=row3[0:128, :])
                w3b = rw.tile([64, 1], F16, tag="w3b")
                nc.sync.dma_start(out=w3b[:], in_=row3[128:192, :])
                robt = []
                if flags["ro_b"]:
                    for i, n in enumerate([768, 384, 192, 1]):
                        w = rw.tile([128, n], F32, tag=f"rob{i}")
                        nc.sync.dma_start(out=w[:], in_=robs[i][:])
                        robt.append(w)

                def transpose_chunk(src_ap, kdim):
                    """src_ap: [128, kdim] fp16 SBUF -> [kdim,128] fp16 SBUF."""
                    pt = pst.tile([128, 128], F16, tag="pt")
                    nc.tensor.transpose(
                        out=pt[:kdim, :], in_=src_ap, identity=ident_t[:]
                    )
                    ct = ro.tile([128, 128], F16, tag="ct")
                    nc.vector.tensor_copy(out=ct[:kdim, :], in_=pt[:kdim, :])
                    return ct

                for gb in range(4):
                    pA = psa.tile([128, 512], F32, tag="pA")
                    pB = psb.tile([128, 256], F32, tag="pB")
                    for f in range(12):
                        li, half = f // 2, f % 2
                        zc = ro.tile([128, 128], F16, tag="zc")
                        nc.sync.dma_start(
                            out=zc[:],
                            in_=z_red[
                                512 * li + 128 * gb: 512 * li + 128 * (gb + 1),
                                128 * half: 128 * (half + 1),
                            ],
                        )
                        zt = transpose_chunk(zc[:], 128)
                        nc.tensor.matmul(
                            out=pA[:], lhsT=zt[:], rhs=w0t[f][:, 0:512],
                            start=(f == 0), stop=(f == 11),
                        )
                        nc.tensor.matmul(
                            out=pB[:], lhsT=zt[:], rhs=w0t[f][:, 512:768],
                            start=(f == 0), stop=(f == 11),
                        )
                    z1 = ro.tile([128, 768], F16, tag="z1")
                    if flags["ro_b"]:
                        nc.vector.tensor_tensor(
                            out=pA[:], in0=pA[:], in1=robt[0][:, 0:512],
                            op=ALU.add,
                        )
                        nc.vector.tensor_tensor(
                            out=pB[:], in0=pB[:], in1=robt[0][:, 512:768],
                            op=ALU.add,
                        )
                    nc.scalar.activation(z1[:, 0:512], pA[:], ACTF.Gelu)
                    nc.scalar.activation(z1[:, 512:768], pB[:], ACTF.Gelu)

                    p2 = psa.tile([128, 384], F32, tag="p2")
                    for f in range(6):
                        zt = transpose_chunk(z1[:, 128 * f:128 * (f + 1)], 128)
                        nc.tensor.matmul(
                            out=p2[:], lhsT=zt[:], rhs=w1t[f][:],
                            start=(f == 0), stop=(f == 5),
                        )
                    if flags["ro_b"]:
                        nc.vector.tensor_tensor(
                            out=p2[:], in0=p2[:], in1=robt[1][:], op=ALU.add
                        )
                    z2 = ro.tile([128, 384], F16, tag="z2")
                    nc.scalar.activation(z2[:], p2[:], ACTF.Gelu)

                    p3 = psb.tile([128, 192], F32, tag="p3")
                    for f in range(3):
                        zt = transpose_chunk(z2[:, 128 * f:128 * (f + 1)], 128)
                        nc.tensor.matmul(
                            out=p3[:], lhsT=zt[:], rhs=w2t[f][:],
                            start=(f == 0), stop=(f == 2),
                        )
                    if flags["ro_b"]:
                        nc.vector.tensor_tensor(
                            out=p3[:], in0=p3[:], in1=robt[2][:], op=ALU.add
                        )
                    z3 = ro.tile([128, 192], F16, tag="z3")
                    nc.scalar.activation(z3[:], p3[:], ACTF.Gelu)

                    po = pso.tile([128, 1], F32, tag="po")
                    zt = transpose_chunk(z3[:, 0:128], 128)
                    nc.tensor.matmul(
                        out=po[:], lhsT=zt[:], rhs=w3a[:],
                        start=True, stop=False,
                    )
                    zt = transpose_chunk(z3[:, 128:192], 64)
                    nc.tensor.matmul(
                        out=po[:], lhsT=zt[:64, :], rhs=w3b[:],
                        start=False, stop=True,
                    )
                    oc = ro.tile([128, 1], F32, tag="oc")
                    if flags["ro_b"]:
                        nc.vector.tensor_tensor(
                            out=po[:], in0=po[:], in1=robt[3][:], op=ALU.add
                        )
                    nc.vector.tensor_copy(out=oc[:], in_=po[:])
                    nc.sync.dma_start(
                        out=out[128 * gb:128 * (gb + 1), :], in_=oc[:]
                    )

    nc.compile()
    return nc


# ----------------------------------------------------------------------------
# entry point
# ----------------------------------------------------------------------------

def kernel(**inputs):
    in_maps, flags = _prep(inputs)
    key = tuple(sorted(flags.items()))
    if key not in _prog_cache:
        _prog_cache[key] = _build(flags)
    nc = _prog_cache[key]

    kwargs = {}
    if TRACE:
        kwargs = dict(trace=True, trace_cores=TRACE_CORES)
    res = run_bass_kernel_spmd(nc, in_maps, list(range(NC)), **kwargs)
    LAST_RESULT["exec_time_ns"] = getattr(res, "exec_time_ns", None)
    LAST_RESULT["res"] = res
    return np.asarray(res.results[0]["out"], dtype=np.float32)


# revision 27
# speedup vs baseline: 1.0000x; 1.0000x over previous
"""Trainium2 Bass kernel for GCNNetwork (GENConv message passing, L=6).

Graph-data parallel over 8 NeuronCores; v2 (fp16 compute pipeline).

 - Nodes sharded contiguously: core c owns rows [7500c, 7500c+7500), re-binned
   into 61 blocks of 128 slots so every block has <=256 in-edges (2 edge tiles).
 - h and y live in SBUF for the whole kernel (fp16); only y is spilled to DRAM
   for the AllGather (split in two halves so the first overlaps LN of the
   second half of blocks).
 - Per edge-tile-pair (one block): one indirect gather of y[src] ([128,2,256]
   fp16), msg = relu(ysrc + ea*wlw) on Act, w = exp(msg-4) on Act (shift keeps
   fp16 in range; cancels in the softmax ratio), ev1 = msg*w on DVE, then a
   0/1-indicator matmul accumulates [denom | numer] into PSUM (fp16 inputs,
   fp32 accumulate).
 - Block post: agg = numer/max(denom,1e-3) + y (DVE divide), transpose, conv
   matmul (fp16 weights), relu -> h (SBUF), pooling indicator matmul.
 - Pool windows are scattered into a [3200,256] fp16 z buffer, AllReduced,
   readout MLP computed replicated on every core in fp16 (fp32 accumulates).

All weights replicated; indicator matrices (st/ind), initial embeddings h0,
and fp16 weight copies are precomputed on the host.
"""
import sys
import numpy as np

for _p in ("/opt/trn_rl_repo", "/root/.axon_site/_ro/trn_rl_repo"):
    if _p not in sys.path:
        sys.path.append(_p)

import concourse.bass as bass
import concourse.bacc as bacc
import concourse.mybir as mybir
import concourse.tile as tile
from concourse.bass_utils import run_bass_kernel_spmd

F32 = mybir.dt.float32
F16 = mybir.dt.float16
I32 = mybir.dt.int32
ALU = mybir.AluOpType
ACTF = mybir.ActivationFunctionType

N, E, B, D, L = 60000, 120000, 512, 256, 6
NTYPES = 25
LN_EPS = 1e-5
EXP_SHIFT = -4.0          # w = exp(msg + EXP_SHIFT); cancels in softmax ratio
DEN_CLAMP = 1e-4          # ln(denom + eps) bias; << exp(EXP_SHIFT) = 0.018
NC = 8
NPC = N // NC             # 7500 real nodes per core
NBLK = 61                 # 128-slot node blocks per core (bin-packed)
NSHARD = NBLK * 128       # 7808 slots per core
TPB = 2                   # edge tiles per block (asserted in prep)
T = NBLK * TPB            # 122 edge tiles per core
GRP = 4                   # blocks per indirect-gather group
ZROWS = 3200              # z buffer rows (L*512 = 3072 used, row 3072 = dump)
ZDUMP = 3072

# module-level knobs (test.py pokes these; harness uses defaults)
TRACE = False
TRACE_CORES = None
LAST_RESULT = {}
CCE_ADD = True            # fuse ea*wlw + y[src] via gather compute_op
DEBUG = False             # add layer-0 intermediate dumps as outputs

_prog_cache = {}


# ----------------------------------------------------------------------------
# host-side preprocessing
# ----------------------------------------------------------------------------

def _glob_row(slot):
    """Global slot -> row in y_full (plain concatenation by core)."""
    return slot


def _prep(inputs):
    x = np.asarray(inputs["x"]).astype(np.int32).reshape(-1)
    ei = np.asarray(inputs["edge_index"]).astype(np.int64)
    ea = np.asarray(inputs["edge_attr"]).astype(np.float32).reshape(-1)
    batch = np.asarray(inputs["batch"]).astype(np.int64).reshape(-1)
    src_all, dst_all = ei[0], ei[1]

    # ---- pass 1: per-core node permutation (bin-pack by in-degree) ----
    glob_slot = np.zeros(N, dtype=np.int64)        # node id -> global slot
    slot_node = []                                 # per core: slot -> node id
    bin_cnts = np.zeros((NC, NBLK), dtype=np.int64)
    for c in range(NC):
        lo, hi = c * NPC, (c + 1) * NPC
        deg = np.bincount(dst_all[(dst_all >= lo) & (dst_all < hi)] - lo,
                          minlength=NPC)
        order = np.argsort(-deg, kind="stable")    # local ids, degree desc
        i = np.arange(NPC)
        chunk, pos = i // NBLK, i % NBLK
        bins = np.where(chunk % 2 == 0, pos, NBLK - 1 - pos)
        cnt = np.zeros(NBLK, dtype=np.int64)
        for b in range(NBLK):
            cnt[b] = deg[order[bins == b]].sum()
        bin_cnts[c] = cnt
        sn = np.full(NSHARD, -1, dtype=np.int64)
        sl = bins * 128 + chunk                    # slot per order-position
        sn[sl] = order + lo
        slot_node.append(sn)
        loc_slot = np.empty(NPC, dtype=np.int64)
        loc_slot[order] = sl
        glob_slot[lo:hi] = c * NSHARD + loc_slot

    assert bin_cnts.max() <= TPB * 128, f"block overflow: {bin_cnts.max()}"

    # ---- pass 2: per-core edge arrays / node arrays in slot order ----
    in_maps = []
    dst_slot_all = glob_slot[dst_all]              # global slot of dst
    node_emb = np.asarray(inputs["node_emb"]).astype(np.float32)
    h0_all = node_emb[x]                           # [N, D]
    colidx = np.arange(128, dtype=np.int64)
    for c in range(NC):
        sel = (dst_slot_all >= c * NSHARD) & (dst_slot_all < (c + 1) * NSHARD)
        ds = dst_slot_all[sel] - c * NSHARD        # local slot of dst
        s = src_all[sel]
        a = ea[sel]
        blk = ds >> 7
        o = np.argsort(blk, kind="stable")
        ds, s, a, blk = ds[o], s[o], a[o], blk[o]
        cnts = np.bincount(blk, minlength=NBLK)
        assert np.all(cnts == bin_cnts[c])
        ne = len(ds)
        bstart = np.concatenate([[0], np.cumsum(cnts)])
        rank = np.arange(ne) - np.repeat(bstart[:-1], cnts)
        slot = blk * (TPB * 128) + rank            # edge slot (tile-major)
        esrc = np.zeros(T * 128, dtype=np.int32)
        dstl = np.full(T * 128, -1, dtype=np.int64)
        eav = np.zeros(T * 128, dtype=np.float32)
        esrc[slot] = _glob_row(glob_slot[s]).astype(np.int32)
        dstl[slot] = ds & 127
        eav[slot] = a
        esrc_pm = esrc.reshape(T, 128).T.copy()
        ea_pm = eav.reshape(T, 128).T.copy()
        # st indicator [128 edge, T, 128]: st[p, t, q] = (dstl[t*128+p]==q)
        dst_pm = dstl.reshape(T, 128).T            # [128, T]
        st_pm = (dst_pm[:, :, None] == colidx[None, None, :]).astype(
            np.float16).copy()

        # ---- node arrays in slot order ----
        sn = slot_node[c]
        valid = sn >= 0
        g0 = int(batch[c * NPC])
        bl = np.full(NSHARD, -1, dtype=np.int64)
        bl[valid] = batch[sn[valid]] - g0
        assert bl.max() < 128, "graph window exceeds 128 per core"
        ind_pm = (bl.reshape(NBLK, 128).T[:, :, None] ==
                  colidx[None, None, :]).astype(np.float16).copy()
        h0 = np.zeros((NSHARD, D), dtype=np.float16)
        h0[valid] = h0_all[sn[valid]].astype(np.float16)
        h0_pm = h0.reshape(NBLK, 128, D).transpose(1, 0, 2).copy()
        zrow_pm = np.zeros((128, L), dtype=np.int32)
        g = g0 + np.arange(128)
        for i in range(L):
            zrow_pm[:, i] = np.where(g < B, 512 * i + g, ZDUMP)

        in_maps.append(
            dict(esrc=esrc_pm, eaf=ea_pm, st=st_pm, ind=ind_pm,
                 h0=h0_pm, zrow=zrow_pm)
        )

    # ---- shared weights (fp16) ----
    wl_w = np.asarray(inputs["wl_w"]).astype(np.float32)      # [L,1,D]
    conv_w = np.asarray(inputs["conv_w"]).astype(np.float32)  # [L,D,D]
    ln_scale = np.asarray(inputs["ln_scale"]).astype(np.float32)
    ln_bias = np.asarray(inputs["ln_bias"]).astype(np.float32)
    wl_b = np.asarray(inputs["wl_b"]).astype(np.float32)
    conv_b = np.asarray(inputs["conv_b"]).astype(np.float32)
    ro_w = [np.asarray(inputs[f"ro_w{i}"]).astype(np.float32) for i in range(4)]
    ro_b = [np.asarray(inputs[f"ro_b{i}"]).astype(np.float32) for i in range(4)]

    flags = dict(
        ln_affine=not (np.all(ln_scale == 1.0) and np.all(ln_bias == 0.0)),
        wl_b=bool(np.any(wl_b != 0.0)),
        conv_b=bool(np.any(conv_b != 0.0)),
        ro_b=any(np.any(b != 0.0) for b in ro_b),
    )

    shared = dict(
        wlw=np.repeat(wl_w.reshape(L, 1, D), 128, axis=1).astype(np.float16),
        convw=conv_w.astype(np.float16),
        ident=np.eye(128, dtype=np.float16),
        row0=ro_w[0].astype(np.float16), row1=ro_w[1].astype(np.float16),
        row2=ro_w[2].astype(np.float16), row3=ro_w[3].astype(np.float16),
    )
    if flags["ln_affine"]:
        shared["lnsc"] = np.repeat(ln_scale.reshape(L, 1, D), 128, axis=1).copy()
        shared["lnbs"] = np.repeat(ln_bias.reshape(L, 1, D), 128, axis=1).copy()
    if flags["wl_b"]:
        shared["wlb"] = np.repeat(wl_b.reshape(L, 1, D), 128, axis=1).astype(
            np.float16)
    if flags["conv_b"]:
        shared["convb"] = np.repeat(conv_b.reshape(L, 1, D), 128, axis=1).copy()
    if flags["ro_b"]:
        for i, b_ in enumerate(ro_b):
            shared[f"rob{i}"] = np.repeat(b_.reshape(1, -1), 128, axis=0).copy()

    for m in in_maps:
        m.update(shared)
    return in_maps, flags


# ----------------------------------------------------------------------------
# device program
# ----------------------------------------------------------------------------

def _build(flags):
    nc = bacc.Bacc("TRN2", target_bir_lowering=False, debug=False,
                   num_devices=NC)

    # const APs for activation float biases
    for tag, val in (("lneps", LN_EPS), ("eshift", EXP_SHIFT),
                     ("denc", DEN_CLAMP)):
        t = nc.alloc_sbuf_tensor(f"const-float32-{tag}", [128, 1], F32)
        nc.gpsimd.memset(t.ap(), val)
        nc.const_aps.aps[(F32, val)] = t.ap()
    nc.all_engine_barrier()

    # inputs
    esrc = nc.dram_tensor("esrc", [128, T], I32, kind="ExternalInput")
    eaf = nc.dram_tensor("eaf", [128, T], F32, kind="ExternalInput")
    st = nc.dram_tensor("st", [128, T, 128], F16, kind="ExternalInput")
    ind = nc.dram_tensor("ind", [128, NBLK, 128], F16, kind="ExternalInput")
    h0 = nc.dram_tensor("h0", [128, NBLK, D], F16, kind="ExternalInput")
    zrow = nc.dram_tensor("zrow", [128, L], I32, kind="ExternalInput")
    wlw = nc.dram_tensor("wlw", [L, 128, D], F16, kind="ExternalInput")
    convw = nc.dram_tensor("convw", [L, D, D], F16, kind="ExternalInput")
    ident = nc.dram_tensor("ident", [128, 128], F16, kind="ExternalInput")
    row0 = nc.dram_tensor("row0", [6 * D, 768], F16, kind="ExternalInput")
    row1 = nc.dram_tensor("row1", [768, 384], F16, kind="ExternalInput")
    row2 = nc.dram_tensor("row2", [384, 192], F16, kind="ExternalInput")
    row3 = nc.dram_tensor("row3", [192, 1], F16, kind="ExternalInput")
    lnsc = lnbs = wlb = convb = None
    if flags["ln_affine"]:
        lnsc = nc.dram_tensor("lnsc", [L, 128, D], F32, kind="ExternalInput")
        lnbs = nc.dram_tensor("lnbs", [L, 128, D], F32, kind="ExternalInput")
    if flags["wl_b"]:
        wlb = nc.dram_tensor("wlb", [L, 128, D], F16, kind="ExternalInput")
    if flags["conv_b"]:
        convb = nc.dram_tensor("convb", [L, 128, D], F32, kind="ExternalInput")
    robs = None
    if flags["ro_b"]:
        robs = [
            nc.dram_tensor(f"rob{i}", [128, n], F32, kind="ExternalInput")
            for i, n in enumerate([768, 384, 192, 1])
        ]

    out = nc.dram_tensor("out", [B, 1], F32, kind="ExternalOutput")
    if DEBUG:
        dbg_y = nc.dram_tensor("dbg_y", [128, NBLK, D], F16,
                               kind="ExternalOutput")
        dbg_pre = nc.dram_tensor("dbg_pre", [128, GRP * TPB, D], F16,
                                 kind="ExternalOutput")
        dbg_ev = nc.dram_tensor("dbg_ev", [128, GRP * TPB, 2 * D], F16,
                                kind="ExternalOutput")
        dbg_h = nc.dram_tensor("dbg_h", [128, NBLK, D], F16,
                               kind="ExternalOutput")

    with tile.TileContext(nc) as tc:
        with (
            tc.tile_pool(name="dram", bufs=1, space="DRAM") as dram,
            tc.tile_pool(name="consts", bufs=1) as cpool,
            tc.tile_pool(name="lweights", bufs=2) as wpool,
        ):
            y_c = dram.tile([NSHARD, D], F16, tag="y_c")
            y_fulls = [
                dram.tile([NC * NSHARD, D], F16, tag=f"y_full_{i}",
                          name=f"y_full_{i}", addr_space="Shared")
                for i in range(L)
            ]
            z_all = dram.tile([ZROWS, D], F16, tag="z_all")
            z_red = dram.tile([ZROWS, D], F16, tag="z_red", addr_space="Shared")

            ident_t = cpool.tile([128, 128], F16, tag="ident")
            nc.sync.dma_start(out=ident_t[:], in_=ident[:])
            esrc_sb = cpool.tile([128, T], I32, tag="esrc_sb")
            nc.sync.dma_start(out=esrc_sb[:], in_=esrc[:])
            ea_sb = cpool.tile([128, T], F32, tag="ea_sb")
            nc.sync.dma_start(out=ea_sb[:], in_=eaf[:])
            zrow_sb = cpool.tile([128, L], I32, tag="zrow_sb")
            nc.sync.dma_start(out=zrow_sb[:], in_=zrow[:])
            st_sb = cpool.tile([128, T, 128], F16, tag="st_sb")
            nc.sync.dma_start(out=st_sb[:], in_=st[:])
            ind_sb = cpool.tile([128, NBLK, 128], F16, tag="ind_sb")
            nc.sync.dma_start(out=ind_sb[:], in_=ind[:])
            h_sb = cpool.tile([128, NBLK, D], F16, tag="h_sb")
            nc.sync.dma_start(out=h_sb[:], in_=h0[:])
            y_sb = cpool.tile([128, NBLK, D], F16, tag="y_sb")

            # ---------- zero z_all ----------
            with tc.tile_pool(name="zz", bufs=1) as zz:
                zt = zz.tile([128, ZROWS // 128, D], F16)
                nc.vector.memset(zt[:], 0.0)
                nc.sync.dma_start(
                    out=z_all[:].rearrange("(k p) d -> p k d", p=128),
                    in_=zt[:],
                )

            # ---------- layers ----------
            with (
                tc.tile_pool(name="ln", bufs=3) as lp,
                tc.tile_pool(name="edge", bufs=4) as xp,
                tc.tile_pool(name="blk", bufs=3) as bp,
                tc.tile_pool(name="ps_nd", bufs=2, space="PSUM") as ps_nd,
                tc.tile_pool(name="ps_xt", bufs=2, space="PSUM") as ps_xt,
                tc.tile_pool(name="ps_h", bufs=2, space="PSUM") as ps_h,
                tc.tile_pool(name="ps_pool", bufs=1, space="PSUM") as ps_pool,
            ):
                for li in range(L):
                    wlw_t = wpool.tile([128, D], F16, tag="wlw")
                    nc.sync.dma_start(out=wlw_t[:], in_=wlw[li])
                    cw_t = wpool.tile([128, 2, D], F16, tag="cw")
                    nc.sync.dma_start(
                        out=cw_t[:],
                        in_=convw[li].rearrange("(c p) d -> p c d", p=128),
                    )
                    if flags["ln_affine"]:
                        lnsc_t = wpool.tile([128, D], F32, tag="lnsc")
                        nc.sync.dma_start(out=lnsc_t[:], in_=lnsc[li])
                        lnbs_t = wpool.tile([128, D], F32, tag="lnbs")
                        nc.sync.dma_start(out=lnbs_t[:], in_=lnbs[li])
                    if flags["wl_b"]:
                        wlb_t = wpool.tile([128, D], F16, tag="wlb")
                        nc.sync.dma_start(out=wlb_t[:], in_=wlb[li])
                    if flags["conv_b"]:
                        convb_t = wpool.tile([128, D], F32, tag="convb")
                        nc.sync.dma_start(out=convb_t[:], in_=convb[li])

                    # ---- LayerNorm: h_sb -> y_sb (and y_c for AllGather) ----
                    for b in range(NBLK):
                        stats = lp.tile([128, 6], F32, tag="stats")
                        nc.vector.bn_stats(stats[:], h_sb[:, b, :])
                        aggr = lp.tile([128, 2], F32, tag="aggr")
                        nc.vector.bn_aggr(aggr[:], stats[:])
                        sd = lp.tile([128, 2], F32, tag="sd")
                        nc.scalar.activation(
                            sd[:, 0:1], aggr[:, 1:2], ACTF.Sqrt, bias=LN_EPS
                        )
                        rs = lp.tile([128, 1], F32, tag="rs")
                        nc.vector.reciprocal(rs[:], sd[:, 0:1])
                        nc.vector.tensor_scalar(
                            out=y_sb[:, b, :], in0=h_sb[:, b, :],
                            scalar1=aggr[:, 0:1], scalar2=rs[:],
                            op0=ALU.subtract, op1=ALU.mult,
                        )
                        if flags["ln_affine"]:
                            nc.vector.tensor_tensor(
                                out=y_sb[:, b, :], in0=y_sb[:, b, :],
                                in1=lnsc_t[:], op=ALU.mult,
                            )
                            nc.vector.tensor_tensor(
                                out=y_sb[:, b, :], in0=y_sb[:, b, :],
                                in1=lnbs_t[:], op=ALU.add,
                            )
                        # spill y to DRAM in 4 batches for the collective
                        if b in (15, 30, 45, 60):
                            lo = {15: 0, 30: 16, 45: 31, 60: 46}[b]
                            rows = slice(lo * 128, (b + 1) * 128)
                            nc.sync.dma_start(
                                out=y_c[rows, :].rearrange(
                                    "(j p) d -> p j d", p=128),
                                in_=y_sb[:, lo:b + 1, :],
                            )

                    if DEBUG and li == 0:
                        nc.sync.dma_start(out=dbg_y[:], in_=y_sb[:])

                    # ---- AllGather y ----
                    y_full = y_fulls[li]
                    nc.gpsimd.collective_compute(
                        "AllGather", ALU.bypass,
                        replica_groups=[list(range(NC))],
                        ins=[y_c[:].opt()],
                        outs=[y_full[:].opt()],
                    )

                    # ---- edges + conv + pool (groups of GRP blocks) ----
                    ppool = ps_pool.tile([128, D], F32, tag="ppool")
                    for g0 in range(0, NBLK, GRP):
                        gn = min(GRP, NBLK - g0)
                        nt = gn * TPB
                        # pre = ea*wlw, then gather-accumulate y[src] on top
                        pre = xp.tile([128, GRP * TPB, D], F16, tag="pre")
                        for k in range(nt):
                            tt = TPB * g0 + k
                            nc.vector.tensor_scalar(
                                out=pre[:, k, :], in0=wlw_t[:],
                                scalar1=ea_sb[:, tt:tt + 1], scalar2=None,
                                op0=ALU.mult,
                            )
                        if flags["wl_b"]:
                            nc.vector.tensor_tensor(
                                out=pre[:, :nt, :], in0=pre[:, :nt, :],
                                in1=wlb_t[:, None, :].to_broadcast(
                                    [128, nt, D]),
                                op=ALU.add,
                            )
                        ysrc = xp.tile([128, GRP * TPB, D], F16, tag="ysrc")
                        for k in range(nt):
                            tt = TPB * g0 + k
                            nc.gpsimd.indirect_dma_start(
                                out=ysrc[:, k, :], out_offset=None,
                                in_=y_full[:],
                                in_offset=bass.IndirectOffsetOnAxis(
                                    ap=esrc_sb[:, tt:tt + 1], axis=0,
                                ),
                            )
                        nc.vector.tensor_tensor(
                            out=pre[:, :nt, :], in0=pre[:, :nt, :],
                            in1=ysrc[:, :nt, :], op=ALU.add,
                        )
                        msg = xp.tile([128, GRP * TPB, D], F16, tag="msg")
                        nc.vector.tensor_scalar(
                            out=msg[:, :nt, :], in0=pre[:, :nt, :],
                            scalar1=0.0, scalar2=None, op0=ALU.max,
                        )
                        ev = xp.tile([128, GRP * TPB, 2 * D], F16, tag="ev")
                        nc.scalar.activation(
                            ev[:, :nt, 0:D], msg[:, :nt, :], ACTF.Exp,
                            bias=EXP_SHIFT,
                        )
                        nc.vector.tensor_tensor(
                            out=ev[:, :nt, D:2 * D], in0=msg[:, :nt, :],
                            in1=ev[:, :nt, 0:D], op=ALU.mult,
                        )
                        if DEBUG and li == 0 and g0 == 0:
                            nc.sync.dma_start(out=dbg_pre[:], in_=pre[:])
                            nc.sync.dma_start(out=dbg_ev[:], in_=ev[:])
                        for k in range(gn):
                            b = g0 + k
                            nd = ps_nd.tile([128, 2 * D], F32, tag="nd")
                            for j in range(TPB):
                                nc.tensor.matmul(
                                    out=nd[:],
                                    lhsT=st_sb[:, TPB * b + j, :],
                                    rhs=ev[:, TPB * k + j, :],
                                    start=(j == 0), stop=(j == TPB - 1),
                                )
                            # block post: agg = numer/denom + y, conv, pool
                            # 1/denom via exp(-ln(denom + eps)) on Act
                            num16 = bp.tile([128, D], F16, tag="num16")
                            nc.scalar.activation(
                                num16[:], nd[:, D:2 * D], ACTF.Copy)
                            lnd = bp.tile([128, D], F16, tag="lnd")
                            nc.scalar.activation(
                                lnd[:], nd[:, 0:D], ACTF.Ln, bias=DEN_CLAMP)
                            rec = bp.tile([128, D], F16, tag="rec")
                            nc.scalar.activation(
                                rec[:], lnd[:], ACTF.Exp, scale=-1.0)
                            xv = bp.tile([128, D], F16, tag="xv")
                            nc.vector.tensor_tensor(
                                out=xv[:], in0=num16[:], in1=rec[:],
                                op=ALU.mult,
                            )
                            xva = bp.tile([128, D], F16, tag="xva")
                            nc.vector.tensor_tensor(
                                out=xva[:], in0=xv[:], in1=y_sb[:, b, :],
                                op=ALU.add,
                            )
                            pxt = ps_xt.tile([128, D], F16, tag="pxt")
                            nc.tensor.transpose(
                                out=pxt[:, 0:128], in_=xva[:, 0:128],
                                identity=ident_t[:],
                            )
                            nc.tensor.transpose(
                                out=pxt[:, 128:256], in_=xva[:, 128:256],
                                identity=ident_t[:],
                            )
                            xts = bp.tile([128, D], F16, tag="xts")
                            nc.vector.tensor_copy(out=xts[:], in_=pxt[:])
                            ph = ps_h.tile([128, D], F32, tag="ph")
                            for c in range(2):
                                nc.tensor.matmul(
                                    out=ph[:],
                                    lhsT=xts[:, 128 * c:128 * (c + 1)],
                                    rhs=cw_t[:, c, :],
                                    start=(c == 0), stop=(c == 1),
                                )
                            if flags["conv_b"]:
                                nc.vector.tensor_tensor(
                                    out=ph[:], in0=ph[:], in1=convb_t[:],
                                    op=ALU.add,
                                )
                            nc.scalar.activation(
                                h_sb[:, b, :], ph[:], ACTF.Relu)
                            nc.tensor.matmul(
                                out=ppool[:], lhsT=ind_sb[:, b, :],
                                rhs=h_sb[:, b, :],
                                start=(b == 0), stop=(b == NBLK - 1),
                            )
                    if DEBUG and li == 0:
                        nc.sync.dma_start(out=dbg_h[:], in_=h_sb[:])
                    # pool -> z_all
                    zp = bp.tile([128, D], F16, tag="zp")
                    nc.scalar.activation(zp[:], ppool[:], ACTF.Copy)
                    nc.gpsimd.indirect_dma_start(
                        out=z_all[:],
                        out_offset=bass.IndirectOffsetOnAxis(
                            ap=zrow_sb[:, li:li + 1], axis=0
                        ),
                        in_=zp[:], in_offset=None,
                    )

            # ---------- AllReduce z ----------
            nc.gpsimd.collective_compute(
                "AllReduce", ALU.add,
                replica_groups=[list(range(NC))],
                ins=[z_all[:].opt()], outs=[z_red[:].opt()],
            )

            # ---------- readout MLP (replicated, fp16) ----------
            with (
                tc.tile_pool(name="row", bufs=1) as rw,
                tc.tile_pool(name="ro", bufs=2) as ro,
                tc.tile_pool(name="ps_a", bufs=1, space="PSUM") as psa,
                tc.tile_pool(name="ps_b", bufs=1, space="PSUM") as psb,
                tc.tile_pool(name="ps_t", bufs=2, space="PSUM") as pst,
                tc.tile_pool(name="ps_o", bufs=1, space="PSUM") as pso,
            ):
                w0t = []
                for f in range(12):
                    w = rw.tile([128, 768], F16, tag=f"w0_{f}")
                    nc.sync.dma_start(out=w[:], in_=row0[f * 128:(f + 1) * 128, :])
                    w0t.append(w)
                w1t = []
                for f in range(6):
                    w = rw.tile([128, 384], F16, tag=f"w1_{f}")
                    nc.sync.dma_start(out=w[:], in_=row1[f * 128:(f + 1) * 128, :])
                    w1t.append(w)
                w2t = []
                for f in range(3):
                    w = rw.tile([128, 192], F16, tag=f"w2_{f}")
                    nc.sync.dma_start(out=w[:], in_=row2[f * 128:(f + 1) * 128, :])
                    w2t.append(w)
                w3a = rw.tile([128, 1], F16, tag="w3a")
                nc.sync.dma_start(out=w3a[:], in_=row3[0:128, :])
                w3b = rw.tile([64, 1], F16, tag="w3b")
                nc.sync.dma_start(out=w3b[:], in_=row3[128:192, :])
                robt = []
                if flags["ro_b"]:
                    for i, n in enumerate([768, 384, 192, 1]):
                        w = rw.tile([128, n], F32, tag=f"rob{i}")
                        nc.sync.dma_start(out=w[:], in_=robs[i][:])
                        robt.append(w)

                def transpose_chunk(src_ap, kdim):
                    """src_ap: [128, kdim] fp16 SBUF -> [kdim,128] fp16 SBUF."""
                    pt = pst.tile([128, 128], F16, tag="pt")
                    nc.tensor.transpose(
                        out=pt[:kdim, :], in_=src_ap, identity=ident_t[:]
                    )
                    ct = ro.tile([128, 128], F16, tag="ct")
                    nc.vector.tensor_copy(out=ct[:kdim, :], in_=pt[:kdim, :])
                    return ct

                for gb in range(4):
                    pA = psa.tile([128, 512], F32, tag="pA")
                    pB = psb.tile([128, 256], F32, tag="pB")
                    for f in range(12):
                        li, half = f // 2, f % 2
                        zc = ro.tile([128, 128], F16, tag="zc")
                        nc.sync.dma_start(
                            out=zc[:],
                            in_=z_red[
                                512 * li + 128 * gb: 512 * li + 128 * (gb + 1),
                                128 * half: 128 * (half + 1),
                            ],
                        )
                        zt = transpose_chunk(zc[:], 128)
                        nc.tensor.matmul(
                            out=pA[:], lhsT=zt[:], rhs=w0t[f][:, 0:512],
                            start=(f == 0), stop=(f == 11),
                        )
                        nc.tensor.matmul(
                            out=pB[:], lhsT=zt[:], rhs=w0t[f][:, 512:768],
                            start=(f == 0), stop=(f == 11),
                        )
                    z1 = ro.tile([128, 768], F16, tag="z1")
                    if flags["ro_b"]:
                        nc.vector.tensor_tensor(
                            out=pA[:], in0=pA[:], in1=robt[0][:, 0:512],
                            op=ALU.add,
                        )
                        nc.vector.tensor_tensor(
                            out=pB[:], in0=pB[:], in1=robt[0][:, 512:768],
                            op=ALU.add,
                        )
                    nc.scalar.activation(z1[:, 0:512], pA[:], ACTF.Gelu)
                    nc.scalar.activation(z1[:, 512:768], pB[:], ACTF.Gelu)

                    p2 = psa.tile([128, 384], F32, tag="p2")
                    for f in range(6):
                        zt = transpose_chunk(z1[:, 128 * f:128 * (f + 1)], 128)
                        nc.tensor.matmul(
                            out=p2[:], lhsT=zt[:], rhs=w1t[f][:],
                            start=(f == 0), stop=(f == 5),
                        )
                    if flags["ro_b"]:
                        nc.vector.tensor_tensor(
                            out=p2[:], in0=p2[:], in1=robt[1][:], op=ALU.add
                        )
                    z2 = ro.tile([128, 384], F16, tag="z2")
                    nc.scalar.activation(z2[:], p2[:], ACTF.Gelu)

                    p3 = psb.tile([128, 192], F32, tag="p3")
                    for f in range(3):
                        zt = transpose_chunk(z2[:, 128 * f:128 * (f + 1)], 128)
                        nc.tensor.matmul(
                            out=p3[:], lhsT=zt[:], rhs=w2t[f][:],
                            start=(f == 0), stop=(f == 2),
                        )
                    if flags["ro_b"]:
                        nc.vector.tensor_tensor(
                            out=p3[:], in0=p3[:], in1=robt[2][:], op=ALU.add
                        )
                    z3 = ro.tile([128, 192], F16, tag="z3")
                    nc.scalar.activation(z3[:], p3[:], ACTF.Gelu)

                    po = pso.tile([128, 1], F32, tag="po")
                    zt = transpose_chunk(z3[:, 0:128], 128)
                    nc.tensor.matmul(
                        out=po[:], lhsT=zt[:], rhs=w3a[:],
                        start=True, stop=False,
                    )
                    zt = transpose_chunk(z3[:, 128:192], 64)
                    nc.tensor.matmul(
                        out=po[:], lhsT=zt[:64, :], rhs=w3b[:],
                        start=False, stop=True,
                    )
                    oc = ro.tile([128, 1], F32, tag="oc")
                    if flags["ro_b"]:
                        nc.vector.tensor_tensor(
                            out=po[:], in0=po[:], in1=robt[3][:], op=ALU.add
                        )
                    nc.vector.tensor_copy(out=oc[:], in_=po[:])
                    nc.sync.dma_start(
                        out=out[128 * gb:128 * (gb + 1), :], in_=oc[:]
                    )

    nc.compile()
    return nc


# ----------------------------------------------------------------------------
# entry point
# ----------------------------------------------------------------------------

def kernel(**inputs):
    in_maps, flags = _prep(inputs)
    key = tuple(sorted(flags.items()))
    if key not in _prog_cache:
        _prog_cache[key] = _build(flags)
    nc = _prog_cache[key]

    kwargs = {}
    if TRACE:
        kwargs = dict(trace=True, trace_cores=TRACE_CORES)
    res = run_bass_kernel_spmd(nc, in_maps, list(range(NC)), **kwargs)
    LAST_RESULT["exec_time_ns"] = getattr(res, "exec_time_ns", None)
    LAST_RESULT["res"] = res
    return np.asarray(res.results[0]["out"], dtype=np.float32)
